# revision 1
# baseline (speedup 1.0000x reference)
"""DeepFM forward kernel for 8 Trainium2 NeuronCores (Bass/Tile).

Strategy (data-parallel over batch, per the sharding hint):
  - Batch B=16384 split 8 ways -> 2048 rows/core. Embedding table, fc
    table and MLP weights replicated to every core.
  - Embedding rows fetched with the SWDGE dma_gather custom instruction
    (512B row per index); fc values fetched the same way from a 64-wide
    zero-padded view of fc (256B stride requirement), per-field with the
    field's offset folded into the source access pattern so raw int16
    ids can be used.
  - FM row stats computed in f32 from the gathered rows; the gathered
    blocks are transposed on the PE into feature-major bf16 layout.
  - MLP runs feature-major: h_{l+1}T = relu(W_l.T @ h_lT + b) so every
    layer uses the weights' natural [in, out] layout as lhsT and no
    activation transposes are needed. bf16 inputs, f32 PSUM accumulate.
  - The FM quirk term 0.5*sum_B(rowsum^2 - rowssq) is a GLOBAL scalar:
    phase A computes per-core partials, the host sums 8 floats, phase B
    takes the scalar and produces sigmoid(mlp_y + lin + 0.5*g + bias).
  - Output y[b] f32 [16384, 1].
"""

import os
import numpy as np

# ---- problem constants (hardcoded; kernel.py must be self-contained) ----
TOTAL = 38279
CAT_SIZES = [31360, 6807, 18, 94]
EMB = 128
F = 4
B = 16384
N_CORES = 8
P = 128
FCW = 64                      # fc padded row width (256B stride for gather)
OFFSETS_NP = np.array([0, 31360, 38167, 38185], dtype=np.int32)

_build_cache = {}


def _build(b_loc, n_cores, use_gather=True, cast_dma=True, phase="A"):
    """Build + compile the per-core SPMD program (phase "A" or "B")."""
    import concourse.bass as bass
    import concourse.mybir as mybir
    import concourse.tile as tile
    from concourse import bacc

    f32 = mybir.dt.float32
    bf16 = mybir.dt.bfloat16
    i32 = mybir.dt.int32
    AF = mybir.ActivationFunctionType
    ALU = mybir.AluOpType
    AX = mybir.AxisListType

    NCH = b_loc // P                 # 128-row chunks per core
    GSZ = min(4, NCH)                # chunks per gather group
    NG = NCH // GSZ
    NB = min(512, b_loc)             # matmul moving (batch) width
    NJ = b_loc // NB
    CPJ = NB // P                    # chunks per n-chunk
    NIDX = GSZ * F * P               # embedding indices per gather group
    need_fm = phase == "A"
    need_mlp = phase == "B"

    nc = bacc.Bacc(
        "TRN2",
        target_bir_lowering=False,
        debug=False,
        num_devices=n_cores,
    )

    # ---- DRAM I/O ----
    emb_d = nc.dram_tensor("emb_table", [TOTAL, EMB], f32, kind="ExternalInput").ap()
    x_d = nc.dram_tensor("x", [b_loc, F], f32, kind="ExternalInput").ap()
    if need_fm:
        gpart_d = nc.dram_tensor("gpart", [1, 1], f32, kind="ExternalOutput").ap()
    if need_mlp:
        bias_d = nc.dram_tensor("bias", [1, 1], f32, kind="ExternalInput").ap()
        fc_d = nc.dram_tensor("fc", [TOTAL, 1], f32, kind="ExternalInput").ap()
        W1_d = nc.dram_tensor("W1", [512, 2048], f32, kind="ExternalInput").ap()
        W2_d = nc.dram_tensor("W2", [2048, 1024], f32, kind="ExternalInput").ap()
        W3_d = nc.dram_tensor("W3", [1024, 512], f32, kind="ExternalInput").ap()
        W4_d = nc.dram_tensor("W4", [512, 1], f32, kind="ExternalInput").ap()
        b1_d = nc.dram_tensor("b1", [2048], f32, kind="ExternalInput").ap()
        b2_d = nc.dram_tensor("b2", [1024], f32, kind="ExternalInput").ap()
        b3_d = nc.dram_tensor("b3", [512], f32, kind="ExternalInput").ap()
        b4_d = nc.dram_tensor("b4", [1, 1], f32, kind="ExternalInput").ap()
        ident_d = nc.dram_tensor("ident", [P, P], f32, kind="ExternalInput").ap()
        g_ext_d = nc.dram_tensor("g_ext", [1, 1], f32, kind="ExternalInput").ap()
        y_d = nc.dram_tensor("y", [b_loc, 1], f32, kind="ExternalOutput").ap()

    KT1, MT1 = 512 // P, 2048 // P
    KT2, MT2 = 2048 // P, 1024 // P
    KT3, MT3 = 1024 // P, 512 // P
    KT4 = 512 // P

    with tile.TileContext(nc) as tc:
        with (
            tc.tile_pool(name="const", bufs=1) as const,
            tc.tile_pool(name="gat", bufs=2) as gat,
            tc.tile_pool(name="work", bufs=2) as work,
            tc.tile_pool(name="acts", bufs=1) as acts,
            tc.tile_pool(name="psmm", bufs=3, space="PSUM") as psum_mm,
            tc.tile_pool(name="pstp", bufs=2, space="PSUM") as psum_tp,
            tc.tile_pool(name="psl4", bufs=1, space="PSUM") as psum_l4,
            tc.tile_pool(name="psmisc", bufs=1, space="PSUM") as psum_misc,
        ):
            # ---- raw ids (int32) for the per-(chunk,field) gathers ----
            xw = const.tile([P, NCH, F], f32, tag="xw")
            nc.sync.dma_start(xw[:], x_d.rearrange("(c p) f -> p c f", p=P))
            xi = const.tile([P, NCH, F], i32, tag="xi")
            nc.vector.tensor_copy(xi[:], xw[:])

            if need_mlp:
                ident = const.tile([P, P], f32, tag="ident")
                nc.sync.dma_start(ident[:], ident_d)
                bias_sb = const.tile([1, 1], f32, tag="bias_sb")
                nc.sync.dma_start(bias_sb[:], bias_d)
                b4_sb = const.tile([1, 1], f32, tag="b4_sb")
                nc.sync.dma_start(b4_sb[:], b4_d)
                ones_row = const.tile([1, P], f32, tag="ones_row")
                nc.vector.memset(ones_row[:], 1.0)

                # ---- weights (DMA-cast f32 -> bf16 via SWDGE) ----
                def load_w(dst, src):
                    if cast_dma:
                        nc.gpsimd.dma_start(dst, src)
                    else:
                        stg = work.tile(list(dst.shape), f32, tag="wstage",
                                        name="wstage")
                        nc.sync.dma_start(stg[:], src)
                        nc.vector.tensor_copy(dst, stg[:])

                W1b = [const.tile([P, 2048], bf16, tag=f"w1_{k}", name=f"w1_{k}")
                       for k in range(KT1)]
                for k in range(KT1):
                    load_w(W1b[k][:], W1_d[k * P:(k + 1) * P, :])
                W2b = [const.tile([P, 1024], bf16, tag=f"w2_{k}", name=f"w2_{k}")
                       for k in range(KT2)]
                for k in range(KT2):
                    load_w(W2b[k][:], W2_d[k * P:(k + 1) * P, :])
                W3b = [const.tile([P, 512], bf16, tag=f"w3_{k}", name=f"w3_{k}")
                       for k in range(KT3)]
                for k in range(KT3):
                    load_w(W3b[k][:], W3_d[k * P:(k + 1) * P, :])
                W4b = const.tile([P, KT4], bf16, tag="w4")
                load_w(W4b[:], W4_d.rearrange("(k p) o -> p (k o)", p=P))

                # ---- biases, partition-major per m-tile ----
                b1_sb = const.tile([P, MT1], f32, tag="b1_sb")
                nc.sync.dma_start(b1_sb[:], b1_d.rearrange("(m p) -> p m", p=P))
                b2_sb = const.tile([P, MT2], f32, tag="b2_sb")
                nc.sync.dma_start(b2_sb[:], b2_d.rearrange("(m p) -> p m", p=P))
                b3_sb = const.tile([P, MT3], f32, tag="b3_sb")
                nc.sync.dma_start(b3_sb[:], b3_d.rearrange("(m p) -> p m", p=P))

                # ---- fc gathers: production-shaped [P,1]-index indirect DMA,
                # one per (chunk, field); the field offset goes in
                # element_offset so raw ids are used directly ----
                fcv = const.tile([P, NCH, F], f32, tag="fcv")
                if use_gather:
                    for c in range(NCH):
                        for f in range(F):
                            nc.gpsimd.indirect_dma_start(
                                out=fcv[:, c, f:f + 1],
                                out_offset=None,
                                in_=fc_d,
                                in_offset=bass.IndirectOffsetOnAxis(
                                    ap=xi[:, c, f:f + 1], axis=0
                                ),
                                element_offset=int(OFFSETS_NP[f]),
                            )
                else:
                    nc.vector.memset(fcv[:], 0.25)
                lin = const.tile([P, NCH], f32, tag="lin")
                nc.vector.reduce_sum(out=lin[:], in_=fcv[:], axis=AX.X)

            if need_fm:
                ones_col = const.tile([P, 1], f32, tag="ones_col")
                nc.vector.memset(ones_col[:], 1.0)
                rs4 = const.tile([P, NCH, F], f32, tag="rs4")
                rssq = const.tile([P, NCH], f32, tag="rssq")
            if need_mlp:
                embT = [const.tile([P, b_loc], bf16, tag=f"embT{f}",
                                   name=f"embT{f}") for f in range(F)]

            # ---- embedding gather (+ FM row stats) (+ PE transpose) ----
            for g in range(NG):
                G = gat.tile([P, GSZ * F, EMB], f32, tag="G")
                if use_gather:
                    for cs in range(GSZ):
                        for f in range(F):
                            nc.gpsimd.indirect_dma_start(
                                out=G[:, cs * F + f, :],
                                out_offset=None,
                                in_=emb_d,
                                in_offset=bass.IndirectOffsetOnAxis(
                                    ap=xi[:, g * GSZ + cs, f:f + 1], axis=0
                                ),
                            )
                else:
                    nc.vector.memset(G[:], 0.01)
                if need_fm:
                    nc.vector.reduce_sum(
                        out=rs4[:, g * GSZ:(g + 1) * GSZ, :], in_=G[:], axis=AX.X
                    )
                    # per-chunk sum of squares (square then reduce; the fused
                    # tensor_tensor_reduce op faults the runtime on this stack)
                    for cs in range(GSZ):
                        c = g * GSZ + cs
                        sq = work.tile([P, F * EMB], f32, tag="sqsc")
                        nc.vector.tensor_tensor(
                            out=sq[:],
                            in0=G[:, cs * F:(cs + 1) * F, :],
                            in1=G[:, cs * F:(cs + 1) * F, :],
                            op=ALU.mult,
                        )
                        nc.vector.reduce_sum(
                            out=rssq[:, c:c + 1], in_=sq[:], axis=AX.X
                        )
                if need_mlp:
                    for cs in range(GSZ):
                        c = g * GSZ + cs
                        for f in range(F):
                            tp = psum_tp.tile([P, P], f32, tag="tp")
                            nc.tensor.transpose(tp[:], G[:, cs * F + f, :],
                                                ident[:])
                            nc.vector.tensor_copy(
                                embT[f][:, c * P:(c + 1) * P], tp[:]
                            )

            if need_fm:
                # ---- FM global scalar partial -> gpart ----
                rowsum = const.tile([P, NCH], f32, tag="rowsum")
                nc.vector.reduce_sum(out=rowsum[:], in_=rs4[:], axis=AX.X)
                sosd = const.tile([P, NCH], f32, tag="sosd")
                nc.vector.tensor_tensor(
                    out=sosd[:], in0=rowsum[:], in1=rowsum[:], op=ALU.mult
                )
                nc.vector.tensor_tensor(
                    out=sosd[:], in0=sosd[:], in1=rssq[:], op=ALU.subtract
                )
                pg = const.tile([P, 1], f32, tag="pg")
                nc.vector.reduce_sum(out=pg[:], in_=sosd[:], axis=AX.X)
                gps = psum_misc.tile([1, 1], f32, tag="gps")
                nc.tensor.matmul(
                    gps[:], lhsT=pg[:], rhs=ones_col[:], start=True, stop=True
                )
                g_sb = const.tile([1, 1], f32, tag="g_sb")
                nc.vector.tensor_copy(g_sb[:], gps[:])
                nc.sync.dma_start(gpart_d, g_sb[:])

            if need_mlp:
                # S = 0.5*g + bias + b4  (scalar)
                g_all = const.tile([1, 1], f32, tag="g_all")
                nc.sync.dma_start(g_all[:], g_ext_d)
                S1 = const.tile([1, 1], f32, tag="S1")
                nc.scalar.activation(S1[:], g_all[:], AF.Identity,
                                     bias=bias_sb[:], scale=0.5)
                S2 = const.tile([1, 1], f32, tag="S2")
                nc.scalar.activation(S2[:], S1[:], AF.Identity,
                                     bias=b4_sb[:], scale=1.0)
                # broadcast S to all partitions via K=1 ones-matmul
                Sps = psum_misc.tile([P, 1], f32, tag="Sps")
                nc.tensor.matmul(
                    Sps[:], lhsT=ones_row[:], rhs=S2[:], start=True, stop=True
                )
                Sbc = const.tile([P, 1], f32, tag="Sbc")
                nc.vector.tensor_copy(Sbc[:], Sps[:])
                linS = const.tile([P, NCH], f32, tag="linS")
                nc.vector.tensor_tensor(
                    out=linS[:],
                    in0=lin[:],
                    in1=Sbc[:].to_broadcast([P, NCH]),
                    op=ALU.add,
                )

                # ---- MLP (feature-major) + tail ----
                ysb = const.tile([P, NCH], f32, tag="ysb")
                layers = [
                    (KT1, MT1, W1b, b1_sb, "h1"),
                    (KT2, MT2, W2b, b2_sb, "h2"),
                    (KT3, MT3, W3b, b3_sb, "h3"),
                ]
                for j in range(NJ):
                    jsl = slice(j * NB, (j + 1) * NB)
                    h_prev = [embT[k][:, jsl] for k in range(KT1)]
                    for (KT, MT, Wb, bsb, lname) in layers:
                        h_next = []
                        for m in range(MT):
                            ps = psum_mm.tile([P, NB], f32, tag="mm")
                            for k in range(KT):
                                nc.tensor.matmul(
                                    ps[:],
                                    lhsT=Wb[k][:, m * P:(m + 1) * P],
                                    rhs=h_prev[k],
                                    start=(k == 0),
                                    stop=(k == KT - 1),
                                )
                            t = acts.tile([P, NB], bf16, tag=f"{lname}_{m}",
                                          name=f"{lname}_{m}_{j}")
                            nc.scalar.activation(
                                t[:], ps[:], AF.Relu, bias=bsb[:, m:m + 1]
                            )
                            h_next.append(t[:])
                        h_prev = h_next
                    # final layer (N=1) in batch-on-partition layout + sigmoid
                    for cs in range(CPJ):
                        c = j * CPJ + cs
                        ps4 = psum_l4.tile([P, 1], f32, tag="l4")
                        for k in range(KT4):
                            nc.tensor.matmul(
                                ps4[:],
                                lhsT=h_prev[k][:, cs * P:(cs + 1) * P],
                                rhs=W4b[:, k:k + 1],
                                start=(k == 0),
                                stop=(k == KT4 - 1),
                            )
                        nc.scalar.activation(
                            ysb[:, c:c + 1], ps4[:], AF.Sigmoid,
                            bias=linS[:, c:c + 1]
                        )

                nc.sync.dma_start(y_d.rearrange("(c p) o -> p (c o)", p=P),
                                  ysb[:])

    nc.compile()
    return nc


def _get_program(b_loc, n_cores, **kw):
    key = (b_loc, n_cores, tuple(sorted(kw.items())))
    if key not in _build_cache:
        _build_cache[key] = _build(b_loc, n_cores, **kw)
    return _build_cache[key]


def _wrap_idx(lin_idx):
    """lin_idx [n] int -> [128, n//16] int16 dma_gather index tile:
    tile[p, s] = lin_idx[s*16 + p%16] (16-wrap, replicated for 8 Q7 cores)."""
    n = lin_idx.shape[0]
    wrap = lin_idx.astype(np.int16).reshape(n // 16, 16).T  # [16, n//16]
    return np.ascontiguousarray(np.tile(wrap, (8, 1)))


def make_in_maps(inputs, b_loc, n_cores, phase="A", g_ext=None):
    """Host-side sharding/layout: slice x over batch, build int16 gather
    index tiles and the 256B-stride padded fc view; replicate the rest."""
    x_int = np.asarray(inputs["x"], dtype=np.float32).astype(np.int32)
    NCH = b_loc // P
    GSZ = min(4, NCH)
    NG = NCH // GSZ

    shared = {
        "emb_table": np.ascontiguousarray(
            np.asarray(inputs["emb_table"], np.float32)),
    }
    if phase == "B":
        shared.update({
            "fc": np.ascontiguousarray(np.asarray(inputs["fc"], np.float32)),
            "ident": np.eye(P, dtype=np.float32),
            "bias": np.asarray(inputs["bias"], np.float32).reshape(1, 1),
            "W1": np.ascontiguousarray(np.asarray(inputs["W1"], np.float32)),
            "W2": np.ascontiguousarray(np.asarray(inputs["W2"], np.float32)),
            "W3": np.ascontiguousarray(np.asarray(inputs["W3"], np.float32)),
            "W4": np.ascontiguousarray(np.asarray(inputs["W4"], np.float32)),
            "b1": np.ascontiguousarray(np.asarray(inputs["b1"], np.float32)),
            "b2": np.ascontiguousarray(np.asarray(inputs["b2"], np.float32)),
            "b3": np.ascontiguousarray(np.asarray(inputs["b3"], np.float32)),
            "b4": np.asarray(inputs["b4"], np.float32).reshape(1, 1),
            "g_ext": np.asarray(g_ext, np.float32).reshape(1, 1),
        })

    x = np.ascontiguousarray(np.asarray(inputs["x"], dtype=np.float32))
    in_maps = []
    for c in range(n_cores):
        m = dict(shared)
        m["x"] = np.ascontiguousarray(x[c * b_loc:(c + 1) * b_loc])
        in_maps.append(m)
    return in_maps


def kernel(**inputs) -> np.ndarray:
    from concourse.bass_utils import run_bass_kernel_spmd

    n_cores = N_CORES
    b_loc = B // n_cores
    cores = list(range(n_cores))
    trace = bool(int(os.environ.get("KERNEL_TRACE", "0")))

    # Phase A: per-core FM partial scalar
    ncA = _get_program(b_loc, n_cores, phase="A")
    resA = run_bass_kernel_spmd(
        ncA, make_in_maps(inputs, b_loc, n_cores, phase="A"), core_ids=cores,
        trace=trace,
    )
    g = np.float32(0.0)
    for r in resA.results:
        g = np.float32(g + np.float32(r["gpart"][0, 0]))

    # Phase B: MLP + tail with the all-reduced scalar
    ncB = _get_program(b_loc, n_cores, phase="B")
    resB = run_bass_kernel_spmd(
        ncB, make_in_maps(inputs, b_loc, n_cores, phase="B", g_ext=g),
        core_ids=cores, trace=trace,
    )
    kernel._last_results = (resA, resB)
    a_ns = resA.exec_time_ns
    b_ns = resB.exec_time_ns
    kernel._last_exec_ns = (
        (a_ns or 0) + (b_ns or 0) if (a_ns is not None or b_ns is not None)
        else None
    )
    kernel._last_exec_parts = (a_ns, b_ns)
    out = np.concatenate([r["y"] for r in resB.results], axis=0)
    return out.astype(np.float32)



# revision 9
# speedup vs baseline: 2.2458x; 2.2458x over previous
"""DeepFM forward kernel for 8 Trainium2 NeuronCores (Bass/Tile), v2.

Strategy (data-parallel over batch, per the sharding hint):
  - Batch B=16384 split 8 ways -> 2048 rows/core; tables + weights
    replicated.
  - Host builds, per field, a [size_f, 256]-bf16 table whose rows are
    [emb_row(128) | fc_value | zeros]. One transposed SWDGE dma_gather
    per field then yields the FEATURE-MAJOR activation tile
    embT[e, b] directly (plus the fc value on partition 0 of the
    second 128-block) -- no PE transposes, half the gather traffic of
    the f32 baseline, and 4 DMA instructions instead of 128.
  - FM row stats via ones-vector matmuls (partition-dim reduction on
    the PE, f32 PSUM accumulate): rowsumT/rowssqT [1, NB] per j-tile;
    the global-scalar partials accumulate on DVE and are written out
    as gpart (1 float/core, summed on host = the only collective).
  - MLP runs feature-major in fp8 (E4M3) with DoubleRow perf mode:
    weights are host-cast to fp8 in the interleaved [ki, (g ko), m]
    layout, activations are produced by the scalar engine directly in
    the paired [128, 2, b] layout, so every 256-wide contraction group
    is ONE matmul (2x effective PE throughput vs bf16).
  - Layer 4 (512 -> 1) and the fc linear term share one [1, NB] PSUM
    accumulation group: 4 plain fp8 matmuls with W4 + 4 ones-matmuls
    over the gathered fc rows; ypre = mlp_pre + lin goes to DRAM.
  - Phase B is a trivial kernel: y = sigmoid(ypre + S) with
    S = bias + b4 + 0.5 * sum(gpart) folded on host.
"""

import os
import numpy as np
import ml_dtypes

# ---- problem constants (hardcoded; kernel.py must be self-contained) ----
TOTAL = 38279
CAT_SIZES = [31360, 6807, 18, 94]
EMB = 128
F = 4
B = 16384
N_CORES = 8
P = 128
NB = 512                       # matmul moving width (batch columns)
OFFSETS_NP = np.array([0, 31360, 38167, 38185], dtype=np.int32)

_build_cache = {}


def _build_a(b_loc, n_cores):
    """Phase A: gather + FM partials + fp8 MLP -> ypre, gpart.

    KERNEL_STAGE env (debug bisect): 1=gather 2=+cast/sq 3=+FM 4=+L1
    5=+L2/L3 6=full (default).
    """
    stage = int(os.environ.get("KERNEL_STAGE", "6"))
    import concourse.bass as bass  # noqa: F401
    import concourse.mybir as mybir
    import concourse.tile as tile
    from concourse import bacc

    f32 = mybir.dt.float32
    bf16 = mybir.dt.bfloat16
    fp8 = mybir.dt.float8e4
    i16 = mybir.dt.int16
    AF = mybir.ActivationFunctionType
    ALU = mybir.AluOpType
    AX = mybir.AxisListType
    DR = mybir.MatmulPerfMode.DoubleRow

    NJ = b_loc // NB             # j-tiles
    NIX = b_loc // 16            # idx tile free dim

    nc = bacc.Bacc(
        "TRN2",
        target_bir_lowering=False,
        debug=False,
        num_devices=n_cores,
    )

    # ---- DRAM I/O ----
    tabs = [
        nc.dram_tensor(f"tab{f}", [CAT_SIZES[f], 256], bf16,
                       kind="ExternalInput").ap()
        for f in range(F)
    ]
    ixs = [
        nc.dram_tensor(f"ix{f}", [P, NIX], i16, kind="ExternalInput").ap()
        for f in range(F)
    ]
    w1q_d = nc.dram_tensor("w1q", [P, 4, 2048], fp8, kind="ExternalInput").ap()
    w2q_d = nc.dram_tensor("w2q", [P, 16, 1024], fp8, kind="ExternalInput").ap()
    w3q_d = nc.dram_tensor("w3q", [P, 8, 512], fp8, kind="ExternalInput").ap()
    w4q_d = nc.dram_tensor("w4q", [P, 4], fp8, kind="ExternalInput").ap()
    b1p_d = nc.dram_tensor("b1p", [P, 16], f32, kind="ExternalInput").ap()
    b2p_d = nc.dram_tensor("b2p", [P, 8], f32, kind="ExternalInput").ap()
    b3p_d = nc.dram_tensor("b3p", [P, 4], f32, kind="ExternalInput").ap()
    ypre_d = nc.dram_tensor("ypre", [1, b_loc], f32, kind="ExternalOutput").ap()
    gpart_d = nc.dram_tensor("gpart", [1, 1], f32, kind="ExternalOutput").ap()

    with tile.TileContext(nc) as tc:
        with (
            tc.tile_pool(name="const", bufs=1) as const,
            tc.tile_pool(name="work", bufs=2) as work,
            tc.tile_pool(name="psmm", bufs=3, space="PSUM") as psum_mm,
            tc.tile_pool(name="psfm", bufs=2, space="PSUM") as psum_fm,
            tc.tile_pool(name="psl4", bufs=1, space="PSUM") as psum_l4,
        ):
            # dma_gather ucode lives in the gpsimd "mlp" library
            from concourse import library_config
            nc.gpsimd.load_library(library_config.mlp)

            # ---- constants / weights (HWDGE, overlaps the gathers) ----
            ones_col = const.tile([P, 1], bf16, tag="ones_col")
            nc.vector.memset(ones_col[:], 1.0)
            ix_sb = [const.tile([P, NIX], i16, tag=f"ix{f}", name=f"ix{f}")
                     for f in range(F)]
            for f in range(F):
                nc.sync.dma_start(ix_sb[f][:], ixs[f])
            w1q = const.tile([P, 4, 2048], fp8, tag="w1q")
            nc.sync.dma_start(w1q[:], w1q_d)
            w2q = const.tile([P, 16, 1024], fp8, tag="w2q")
            nc.sync.dma_start(w2q[:], w2q_d)
            w3q = const.tile([P, 8, 512], fp8, tag="w3q")
            nc.sync.dma_start(w3q[:], w3q_d)
            w4q = const.tile([P, 4], fp8, tag="w4q")
            nc.sync.dma_start(w4q[:], w4q_d)
            b1p = const.tile([P, 16], f32, tag="b1p")
            nc.sync.dma_start(b1p[:], b1p_d)
            b2p = const.tile([P, 8], f32, tag="b2p")
            nc.sync.dma_start(b2p[:], b2p_d)
            b3p = const.tile([P, 4], f32, tag="b3p")
            nc.sync.dma_start(b3p[:], b3p_d)

            # ---- transposed gathers: G[f][e, s, b]; s=0 emb dims, s=1 fc ----
            G = [const.tile([P, 2, b_loc], bf16, tag=f"g{f}", name=f"g{f}")
                 for f in range(F)]
            for f in range(F):
                nc.gpsimd.dma_gather(
                    G[f][:], tabs[f], ix_sb[f][:], b_loc, b_loc, 256,
                    transpose=True, single_packet=False,
                )

            # ---- fp8 cast into DoubleRow pair layout (L1 rhs) ----
            PT = [const.tile([P, 2, b_loc], fp8, tag=f"p{g}", name=f"p{g}")
                  for g in range(2)]
            if stage >= 2:
                for f in range(F):
                    nc.vector.tensor_copy(PT[f // 2][:, f % 2, :],
                                          G[f][:, 0, :])
            # full-batch squares for the FM sum-of-squares
            SQ = [const.tile([P, b_loc], bf16, tag=f"sq{f}", name=f"sq{f}")
                  for f in range(F)]
            if stage >= 2:
                for f in range(F):
                    nc.vector.tensor_tensor(
                        out=SQ[f][:], in0=G[f][:, 0, :], in1=G[f][:, 0, :],
                        op=ALU.mult,
                    )

            # ---- activations (full-batch tiles, fp8 pair layout) ----
            H1 = [const.tile([P, 2, b_loc], fp8, tag=f"h1_{g}", name=f"h1_{g}")
                  for g in range(8)]
            H2 = [const.tile([P, 2, b_loc], fp8, tag=f"h2_{g}", name=f"h2_{g}")
                  for g in range(4)]
            H3 = [const.tile([P, 2, b_loc], fp8, tag=f"h3_{g}", name=f"h3_{g}")
                  for g in range(2)]
            ypre_sb = const.tile([1, b_loc], f32, tag="ypre_sb")
            gacc = const.tile([1, NB], f32, tag="gacc")
            nc.vector.memset(gacc[:], 0.0)

            if stage < 6:
                nc.vector.memset(ypre_sb[:], 0.0)
            for j in range(NJ):
                if stage < 3:
                    break
                jsl = slice(j * NB, (j + 1) * NB)
                # ---- FM: rowsumT / rowssqT on the PE ----
                psA = psum_fm.tile([1, NB], f32, tag="psA")
                for f in range(F):
                    nc.tensor.matmul(
                        psA[:], lhsT=ones_col[:], rhs=G[f][:, 0, jsl],
                        start=(f == 0), stop=(f == F - 1),
                    )
                psB = psum_fm.tile([1, NB], f32, tag="psB")
                for f in range(F):
                    nc.tensor.matmul(
                        psB[:], lhsT=ones_col[:], rhs=SQ[f][:, jsl],
                        start=(f == 0), stop=(f == F - 1),
                    )
                rs = work.tile([1, NB], f32, tag="rs", name=f"rs{j}")
                nc.vector.tensor_copy(rs[:], psA[:])
                dd = work.tile([1, NB], f32, tag="dd", name=f"dd{j}")
                nc.vector.tensor_tensor(out=dd[:], in0=rs[:], in1=rs[:],
                                        op=ALU.mult)
                nc.vector.tensor_tensor(out=dd[:], in0=dd[:], in1=psB[:],
                                        op=ALU.subtract)
                nc.vector.tensor_tensor(out=gacc[:], in0=gacc[:], in1=dd[:],
                                        op=ALU.add)

                # ---- MLP layers 1-3, fp8 DoubleRow, feature-major ----
                # (KG groups of 256 contraction, M m-tiles of 128, rhs pairs)
                layers = [
                    (2, 16, w1q, b1p, PT, H1),
                    (8, 8, w2q, b2p, H1, H2),
                    (4, 4, w3q, b3p, H2, H3),
                ]
                if stage < 4:
                    layers = []
                elif stage < 5:
                    layers = layers[:1]
                for (KG, MT, wq, bp, Hin, Hout) in layers:
                    for mt in range(MT):
                        ps = psum_mm.tile([P, NB], f32, tag="mm")
                        for g in range(KG):
                            nc.tensor.matmul(
                                ps[:],
                                lhsT=wq[:, 2 * g:2 * g + 2,
                                        mt * P:(mt + 1) * P],
                                rhs=Hin[g][:, :, jsl],
                                start=(g == 0), stop=(g == KG - 1),
                                perf_mode=DR,
                            )
                        nc.scalar.activation(
                            Hout[mt // 2][:, mt % 2, jsl], ps[:], AF.Relu,
                            bias=bp[:, mt:mt + 1],
                        )

                # ---- L4 (512->1) + fc linear term in one PSUM group ----
                if stage < 6:
                    continue
                ps4 = psum_l4.tile([1, NB], f32, tag="l4")
                for kt in range(4):
                    nc.tensor.matmul(
                        ps4[:], lhsT=w4q[:, kt:kt + 1],
                        rhs=H3[kt // 2][:, kt % 2, jsl],
                        start=(kt == 0), stop=False,
                    )
                for f in range(F):
                    nc.tensor.matmul(
                        ps4[:], lhsT=ones_col[:], rhs=G[f][:, 1, jsl],
                        start=False, stop=(f == F - 1),
                    )
                nc.scalar.activation(ypre_sb[:, jsl], ps4[:], AF.Identity)

            # ---- outputs ----
            gp = const.tile([1, 1], f32, tag="gp")
            nc.vector.reduce_sum(out=gp[:], in_=gacc[:], axis=AX.X)
            nc.sync.dma_start(gpart_d, gp[:])
            nc.sync.dma_start(ypre_d, ypre_sb[:])

    nc.compile()
    return nc


def _build_b(b_loc, n_cores):
    """Phase B: y = sigmoid(ypre + S), S folded on host."""
    import concourse.mybir as mybir
    import concourse.tile as tile
    from concourse import bacc

    f32 = mybir.dt.float32
    AF = mybir.ActivationFunctionType
    NCH = b_loc // P

    nc = bacc.Bacc(
        "TRN2",
        target_bir_lowering=False,
        debug=False,
        num_devices=n_cores,
    )
    yin_d = nc.dram_tensor("yin", [P, NCH], f32, kind="ExternalInput").ap()
    sv_d = nc.dram_tensor("sv", [P, 1], f32, kind="ExternalInput").ap()
    y_d = nc.dram_tensor("y", [b_loc, 1], f32, kind="ExternalOutput").ap()

    with tile.TileContext(nc) as tc:
        with tc.tile_pool(name="const", bufs=1) as const:
            yin = const.tile([P, NCH], f32, tag="yin")
            nc.sync.dma_start(yin[:], yin_d)
            sv = const.tile([P, 1], f32, tag="sv")
            nc.sync.dma_start(sv[:], sv_d)
            ysb = const.tile([P, NCH], f32, tag="ysb")
            nc.scalar.activation(ysb[:], yin[:], AF.Sigmoid, bias=sv[:])
            nc.sync.dma_start(y_d.rearrange("(c p) o -> p (c o)", p=P), ysb[:])

    nc.compile()
    return nc


def _get_program(phase, b_loc, n_cores):
    key = (phase, b_loc, n_cores)
    if key not in _build_cache:
        _build_cache[key] = (
            _build_a(b_loc, n_cores) if phase == "A" else _build_b(b_loc, n_cores)
        )
    return _build_cache[key]


def _wrap_idx(lin_idx):
    """lin_idx [n] int -> [128, n//16] int16 dma_gather index tile:
    tile[p, s] = lin_idx[s*16 + p%16] (16-wrap, replicated for 8 Q7 cores)."""
    n = lin_idx.shape[0]
    wrap = lin_idx.astype(np.int16).reshape(n // 16, 16).T  # [16, n//16]
    return np.ascontiguousarray(np.tile(wrap, (8, 1)))


def _prep_shared(inputs):
    """Host-side table/weight prep shared by all cores."""
    bf = ml_dtypes.bfloat16
    f8 = ml_dtypes.float8_e4m3
    emb16 = np.asarray(inputs["emb_table"], np.float32).astype(bf)  # [T,128]
    fc16 = np.asarray(inputs["fc"], np.float32).astype(bf)          # [T,1]
    tabs = {}
    for f in range(F):
        sz = CAT_SIZES[f]
        off = int(OFFSETS_NP[f])
        t = np.zeros((sz, 256), dtype=bf)
        t[:, :EMB] = emb16[:sz]
        t[:, EMB] = fc16[off:off + sz, 0]
        tabs[f"tab{f}"] = t

    def dr_pack(w, kgroups):
        # [K, M] f32 -> [128, 2*kgroups, M] fp8 with (ki, (g ko), m) layout
        K, M = w.shape
        w = np.asarray(w, np.float32).reshape(kgroups, 2, P, M)
        return np.ascontiguousarray(
            w.transpose(2, 0, 1, 3).reshape(P, 2 * kgroups, M).astype(f8)
        )

    sh = dict(tabs)
    sh["w1q"] = dr_pack(np.asarray(inputs["W1"]), 2)
    sh["w2q"] = dr_pack(np.asarray(inputs["W2"]), 8)
    sh["w3q"] = dr_pack(np.asarray(inputs["W3"]), 4)
    sh["w4q"] = np.ascontiguousarray(
        np.asarray(inputs["W4"], np.float32).reshape(4, P).T.astype(f8)
    )
    for name, mt in (("b1", 16), ("b2", 8), ("b3", 4)):
        sh[f"{name}p"] = np.ascontiguousarray(
            np.asarray(inputs[name], np.float32).reshape(mt, P).T
        )
    return sh


def kernel(**inputs) -> np.ndarray:
    from concourse.bass_utils import run_bass_kernel_spmd

    n_cores = N_CORES
    b_loc = B // n_cores
    cores = list(range(n_cores))
    trace = bool(int(os.environ.get("KERNEL_TRACE", "0")))

    x_int = np.asarray(inputs["x"], np.float32).astype(np.int32)  # [B, F]
    shared = _prep_shared(inputs)

    # Phase A: per-core ypre + FM partial
    ncA = _get_program("A", b_loc, n_cores)
    in_maps = []
    for c in range(n_cores):
        m = dict(shared)
        xs = x_int[c * b_loc:(c + 1) * b_loc]           # [b_loc, F]
        for f in range(F):
            m[f"ix{f}"] = _wrap_idx(xs[:, f])
        in_maps.append(m)
    resA = run_bass_kernel_spmd(ncA, in_maps, core_ids=cores, trace=trace)

    g = np.float32(0.0)
    for r in resA.results:
        g = np.float32(g + np.float32(r["gpart"][0, 0]))
    S = np.float32(
        np.asarray(inputs["bias"], np.float32).reshape(-1)[0]
        + np.asarray(inputs["b4"], np.float32).reshape(-1)[0]
        + 0.5 * g
    )

    # Phase B: y = sigmoid(ypre + S)
    ncB = _get_program("B", b_loc, n_cores)
    sv = np.full((P, 1), S, dtype=np.float32)
    NCH = b_loc // P
    in_maps_b = []
    for c in range(n_cores):
        ypre = np.asarray(resA.results[c]["ypre"], np.float32).reshape(b_loc)
        in_maps_b.append({
            "yin": np.ascontiguousarray(ypre.reshape(NCH, P).T),
            "sv": sv,
        })
    resB = run_bass_kernel_spmd(ncB, in_maps_b, core_ids=cores, trace=trace)

    kernel._last_results = (resA, resB)
    a_ns = resA.exec_time_ns
    b_ns = resB.exec_time_ns
    kernel._last_exec_ns = (
        (a_ns or 0) + (b_ns or 0) if (a_ns is not None or b_ns is not None)
        else None
    )
    kernel._last_exec_parts = (a_ns, b_ns)
    out = np.concatenate([r["y"] for r in resB.results], axis=0)
    return out.astype(np.float32)
